# revision 5
# baseline (speedup 1.0000x reference)
"""Bass/Trainium2 kernel for a 7-step LSTM decoder (nn_Decoder_19705309954222).

    zx_t = x_t @ Wk + b ; z_t = zx_t + h_{t-1} @ Wr
    i,f,g,o = sig/tanh gates(z_t) ; c_t = f*c + i*g ; h_t = o*tanh(c_t)
    output: h_t for t=0..6, shape [B, T, U].

Sharding: data-parallel over 8 NeuronCores (batch 32768 -> 4096/core), weights
replicated. On-device layout is fully transposed: hidden state h^T is [U=256,
batch], kept as ONE SBUF tile [128, 2*BC] with the two 128-row halves of U side
by side in the free dim; gate pre-activations are 4 PSUM tiles [128, 2*BC] (one
per gate). The recurrent matmul keeps Wr/Wk stationary and streams the batch as
the moving operand; the input projection AND the bias (as a constant-1 row on x,
K=36->37) are fused into the same PSUM accumulation group. The host
pre-transposes x/h0/c0 and re-transposes the output so the device does zero
transposes and all DMAs are wide contiguous rows.
"""

import os
import numpy as np
import ml_dtypes

import concourse.bacc as bacc
import concourse.mybir as mybir
import concourse.tile as tile
from concourse.bass_utils import run_bass_kernel_spmd

B, T, F, U = 32768, 7, 36, 256
FK = F + 1  # x augmented with a constant-1 row; Wk augmented with bias row
G = 4 * U  # 1024
N_CORES = 8
BL = B // N_CORES  # 4096 batch rows per core
M_TILES = G // 128  # 8; gate gi covers m = 2*gi (+0/+1)

# dtype / size config (dev override via env; defaults = shipped config)
MM_DT_NAME = os.environ.get("LSTM_MM_DT", "bf16")  # matmul operands: f32|f32r|bf16
GATE_DT_NAME = os.environ.get("LSTM_GATE_DT", "f32")  # i,f,g,o and tanh(c) tiles
CELL_DT_NAME = os.environ.get("LSTM_CELL_DT", "f32")  # c tiles
OUT_DT_NAME = os.environ.get("LSTM_OUT_DT", "f32")  # h tiles / DRAM output
BC = int(os.environ.get("LSTM_BC", "512"))  # batch columns per chunk

_DT = {
    "f32": mybir.dt.float32,
    "f32r": mybir.dt.float32r,
    "bf16": mybir.dt.bfloat16,
}
_NP = {"f32": np.float32, "f32r": np.float32, "bf16": ml_dtypes.bfloat16}

f32 = mybir.dt.float32


def _build_program():
    gate_dt = _DT[GATE_DT_NAME]
    cell_dt = _DT[CELL_DT_NAME]
    out_dt = _DT[OUT_DT_NAME]
    # dtype stored in DRAM/SBUF for matmul inputs (f32r lives as f32 + AP bitcast)
    io_dt = f32 if MM_DT_NAME in ("f32", "f32r") else _DT[MM_DT_NAME]
    h_mm_dt = io_dt  # h feedback into next step's matmul

    nc = bacc.Bacc("TRN2", target_bir_lowering=False, debug=False)

    xT_d = nc.dram_tensor("xT", [T, FK, BL], io_dt, kind="ExternalInput")
    h0T_d = nc.dram_tensor("h0T", [U, BL], io_dt, kind="ExternalInput")
    c0T_d = nc.dram_tensor("c0T", [U, BL], f32, kind="ExternalInput")
    # wr: col block (k*8+m) = Wr[k*128:(k+1)*128, m*128:(m+1)*128]
    wr_d = nc.dram_tensor("wr", [128, 2 * M_TILES * 128], io_dt, kind="ExternalInput")
    wk_d = nc.dram_tensor("wk", [FK, G], io_dt, kind="ExternalInput")
    out_d = nc.dram_tensor("hsT", [T, U, BL], out_dt, kind="ExternalOutput")

    def mm_ap(ap):
        return ap.bitcast(mybir.dt.float32r) if MM_DT_NAME == "f32r" else ap

    n_chunks = BL // BC
    Sig = mybir.ActivationFunctionType.Sigmoid
    Tanh = mybir.ActivationFunctionType.Tanh
    MUL = mybir.AluOpType.mult
    ADD = mybir.AluOpType.add
    GATE_FUNCS = [Sig, Sig, Tanh, Sig]  # i, f, g, o

    with tile.TileContext(nc) as tc:
        with (
            tc.tile_pool(name="w", bufs=1) as wp,
            tc.tile_pool(name="x", bufs=1 if io_dt == f32 else 2) as xp,
            tc.tile_pool(name="state", bufs=2) as sp,
            tc.tile_pool(name="gates", bufs=2) as gp,
            tc.tile_pool(name="ew", bufs=2) as ep,
            tc.tile_pool(name="z", bufs=1, space="PSUM") as zp,
        ):
            wr_t = wp.tile([128, 2 * M_TILES * 128], io_dt, tag="wr")
            nc.sync.dma_start(wr_t[:], wr_d.ap())
            wk_t = wp.tile([FK, G], io_dt, tag="wk")
            nc.sync.dma_start(wk_t[:], wk_d.ap())

            def wr_ap(k, m):
                j = (k * M_TILES + m) * 128
                return mm_ap(wr_t[:, j:j + 128])

            def wk_ap(m):
                return mm_ap(wk_t[:, m * 128:(m + 1) * 128])

            def emit_chunk_load(ci):
                b0 = ci * BC
                par = ci % 2
                x_t = xp.tile([FK, T * BC], io_dt, tag=f"x{par}")
                for t in range(T):
                    nc.sync.dma_start(
                        x_t[:, t * BC:(t + 1) * BC], xT_d.ap()[t, :, b0:b0 + BC]
                    )
                h = sp.tile([128, 2 * BC], h_mm_dt, tag=f"h{par}")
                c = sp.tile([128, 2 * BC], cell_dt, tag=f"c{par}")
                for p in range(2):
                    nc.sync.dma_start(
                        h[:, p * BC:(p + 1) * BC],
                        h0T_d.ap()[p * 128:(p + 1) * 128, b0:b0 + BC],
                    )
                    nc.sync.dma_start(
                        c[:, p * BC:(p + 1) * BC],
                        c0T_d.ap()[p * 128:(p + 1) * 128, b0:b0 + BC],
                    )
                return {"x": x_t, "h": h, "c": c, "b0": b0, "ci": ci}

            def emit_step(st, t):
                ci, b0 = st["ci"], st["b0"]
                par = ci % 2
                x_t, h_prev, c_prev = st["x"], st["h"], st["c"]
                xt_ap = mm_ap(x_t[:, t * BC:(t + 1) * BC])

                # gate pre-activations: 4 PSUM tiles [128, 2*BC]; halves are the
                # two 128-row U-blocks (m = 2*gi + p), each a 3-matmul accum group
                z = []
                for gi in range(4):
                    zg = zp.tile([128, 2 * BC], f32, tag=f"z{gi}")
                    for p in range(2):
                        m = 2 * gi + p
                        zs = zg[:, p * BC:(p + 1) * BC]
                        nc.tensor.matmul(
                            zs, wr_ap(0, m), mm_ap(h_prev[:, 0:BC]),
                            start=True, stop=False,
                        )
                        nc.tensor.matmul(
                            zs, wr_ap(1, m), mm_ap(h_prev[:, BC:2 * BC]),
                            start=False, stop=False,
                        )
                        nc.tensor.matmul(zs, wk_ap(m), xt_ap, start=False, stop=True)
                    z.append(zg)

                gt = []
                for gi in range(4):
                    g_t = gp.tile([128, 2 * BC], gate_dt, tag=f"g{gi}_{par}")
                    nc.scalar.activation(g_t[:], z[gi][:], GATE_FUNCS[gi])
                    gt.append(g_t)
                i_t, f_t, g_t, o_t = gt

                ig = ep.tile([128, 2 * BC], gate_dt, tag=f"ig{par}")
                nc.vector.tensor_tensor(ig[:], i_t[:], g_t[:], MUL)
                cn = sp.tile([128, 2 * BC], cell_dt, tag=f"c{par}")
                nc.vector.tensor_tensor(cn[:], f_t[:], c_prev[:], MUL)
                nc.vector.tensor_tensor(cn[:], cn[:], ig[:], ADD)
                tc_t = ep.tile([128, 2 * BC], gate_dt, tag=f"tc{par}")
                nc.scalar.activation(tc_t[:], cn[:], Tanh)
                hn = sp.tile([128, 2 * BC], out_dt, tag=f"ho{par}")
                nc.vector.tensor_tensor(hn[:], o_t[:], tc_t[:], MUL)
                for p in range(2):
                    nc.sync.dma_start(
                        out_d.ap()[t, p * 128:(p + 1) * 128, b0:b0 + BC],
                        hn[:, p * BC:(p + 1) * BC],
                    )
                if out_dt == h_mm_dt:
                    h_mm = hn
                else:
                    h_mm = sp.tile([128, 2 * BC], h_mm_dt, tag=f"h{par}")
                    nc.vector.tensor_copy(h_mm[:], hn[:])
                st["h"], st["c"] = h_mm, cn

            for pair in range(0, n_chunks, 2):
                stA = emit_chunk_load(pair)
                stB = emit_chunk_load(pair + 1) if pair + 1 < n_chunks else None
                for t in range(T):
                    emit_step(stA, t)
                    if stB is not None:
                        emit_step(stB, t)

    nc.compile()
    return nc


_PROGRAM = None


def _get_program():
    global _PROGRAM
    if _PROGRAM is None:
        _PROGRAM = _build_program()
    return _PROGRAM


def _prep_inputs(x, h0, c0, Wk, Wr, b):
    io_np = _NP[MM_DT_NAME]
    wr_host = np.ascontiguousarray(
        Wr.reshape(2, 128, M_TILES, 128).transpose(1, 0, 2, 3)
        .reshape(128, 2 * M_TILES * 128)
    ).astype(io_np)
    wk_host = np.concatenate([Wk, b[None, :]], axis=0).astype(io_np)  # [37, 1024]
    in_maps = []
    for i in range(N_CORES):
        s = slice(i * BL, (i + 1) * BL)
        xT = np.empty((T, FK, BL), dtype=io_np)
        xT[:, :F, :] = x[s].transpose(1, 2, 0).astype(io_np)
        xT[:, F, :] = np.float32(1.0)
        h0T = np.ascontiguousarray(h0[s].T).astype(io_np)
        c0T = np.ascontiguousarray(c0[s].T).astype(np.float32)
        in_maps.append(
            {"xT": xT, "h0T": h0T, "c0T": c0T, "wr": wr_host, "wk": wk_host}
        )
    return in_maps


def _gather_output(results):
    outs = []
    for i in range(N_CORES):
        hsT = np.asarray(results[i]["hsT"]).astype(np.float32)  # [T, U, BL]
        outs.append(hsT.transpose(2, 0, 1))  # [BL, T, U]
    return np.ascontiguousarray(np.concatenate(outs, axis=0))


def kernel(x, h0, c0, Wk, Wr, b, _trace=False):
    x = np.asarray(x, dtype=np.float32)
    h0 = np.asarray(h0, dtype=np.float32)
    c0 = np.asarray(c0, dtype=np.float32)
    Wk = np.asarray(Wk, dtype=np.float32)
    Wr = np.asarray(Wr, dtype=np.float32)
    b = np.asarray(b, dtype=np.float32)

    nc = _get_program()
    in_maps = _prep_inputs(x, h0, c0, Wk, Wr, b)
    res = run_bass_kernel_spmd(
        nc, in_maps, core_ids=list(range(N_CORES)), trace=_trace
    )
    out = _gather_output(res.results)
    if _trace:
        return out, res
    return out
